# revision 38
# baseline (speedup 1.0000x reference)
# DETR multi-head dot-product attention for Trainium2 (Bass/Tile), 8 NeuronCores.
#
# Problem (hardcoded): B=4, S=1024, D=1024, H=16, HD=64, f32.
#   q = (inputs_q + pos_emb_q) @ wq + bq;  q /= sqrt(HD)
#   k = (inputs_kv + pos_emb_k) @ wk + bk
#   v = (inputs_kv + pos_emb_v) @ wv + bv          (bv == 0 by problem spec)
#   attn = softmax(q k^T + key_padding_bias); out = (attn v) @ wo + bo
#
# Sharding: 8 cores = 4 batches x 2 head-groups of 8 heads. Each core computes
# its batch's projections restricted to its head-group's features (512 of 1024),
# full attention for its 8 heads, and a partial output projection. The host
# sums the two head-group partials per batch.
#
# All matmul operands are bf16 (f32 PSUM accumulation); rel-err budget (2e-2)
# has ~10x headroom over bf16 noise. bf16 removes the f32r small-N penalty,
# which unlocks the cheap AV orientation: lhsT = P^T chunk [s_k,128 s_q],
# rhs = V chunk [s_k, HD+1] -> av[s_q, HD+1] at 65 cycles per chunk matmul
# (vs 512 the other way). Column HD of V is the padding mask, so av[:, HD]
# is the softmax denominator as a per-partition scalar: normalization is a
# reciprocal + per-partition tensor_scalar multiply, no broadcast matmul.
# A bf16 PE transpose restores feature-major x^T for the output projection;
# the Pool engine copies it PSUM->SBUF to keep DVE/ACT free.
#
# Schedule: projections accumulate over feature chunks in PSUM so the PE
# starts ~1.5us in, as soon as the first weight/input chunks land (the
# previous design waited ~22us for whole tensors). Phase order K, V, Q keeps
# the DMA queue aligned with consumption order; attention is software-
# pipelined across (s_q-half, head) slots with exp (ACT) as the pacing
# engine, and the Q-half1 projection + output projections are interleaved
# into the slot stream as PE filler.

import sys

for _p in ("/opt/trn_rl_repo", "/root/.axon_site/_ro/trn_rl_repo"):
    if _p not in sys.path:
        sys.path.append(_p)

import numpy as np

import concourse.bass as bass
import concourse.mybir as mybir
import concourse.tile as tile
from concourse import bacc
from concourse.bass_utils import run_bass_kernel_spmd

B, S, D = 4, 1024, 1024
H, HD = 16, 64
F = 512          # features per head-group core (8 heads * 64)
NH = 8           # heads per core
P = 128          # partitions
KC = D // P      # contraction chunks for the input projections (8)
SC = S // P      # sequence chunks (8)
SH = 512         # S-half
HD1 = HD + 1     # head_dim + denominator column

f32 = mybir.dt.float32
bf16 = mybir.dt.bfloat16


def build_program(repeat=1):
    nc = bacc.Bacc("TRN2", target_bir_lowering=False, debug=False)

    xq_d = nc.dram_tensor("xq", [D, S], bf16, kind="ExternalInput")
    xkv_d = nc.dram_tensor("xkv", [D, S], bf16, kind="ExternalInput")
    pq_d = nc.dram_tensor("pq", [D, S], bf16, kind="ExternalInput")
    pk_d = nc.dram_tensor("pk", [D, S], bf16, kind="ExternalInput")
    pv_d = nc.dram_tensor("pv", [D, S], bf16, kind="ExternalInput")
    wq_d = nc.dram_tensor("wq", [D, F], bf16, kind="ExternalInput")
    wk_d = nc.dram_tensor("wk", [D, F], bf16, kind="ExternalInput")
    wv_d = nc.dram_tensor("wv", [D, F], bf16, kind="ExternalInput")
    wo_d = nc.dram_tensor("wo", [F, D], bf16, kind="ExternalInput")
    bq_d = nc.dram_tensor("bq", [F], f32, kind="ExternalInput")
    bk_d = nc.dram_tensor("bk", [F], f32, kind="ExternalInput")
    bo_d = nc.dram_tensor("bo", [D], f32, kind="ExternalInput")
    mk_d = nc.dram_tensor("mk", [S], f32, kind="ExternalInput")  # padding mask
    # mask replicated per head for V's denominator column
    vones_d = nc.dram_tensor("vones", [P, SC, NH], bf16, kind="ExternalInput")
    ident_d = nc.dram_tensor("ident", [P, P], bf16, kind="ExternalInput")
    out_d = nc.dram_tensor("out_t", [D, S], f32, kind="ExternalOutput")

    with tile.TileContext(nc) as tc:
        with (
            tc.tile_pool(name="persist", bufs=1) as persist,
            tc.tile_pool(name="wmat", bufs=1) as w_pool,
            tc.tile_pool(name="lin", bufs=3) as lin_pool,
            tc.tile_pool(name="raw", bufs=5) as raw_pool,
            tc.tile_pool(name="ptp", bufs=7) as pt_pool,
            tc.tile_pool(name="xnp", bufs=2) as xn_pool,
            tc.tile_pool(name="rp", bufs=2) as r_pool,
            tc.tile_pool(name="outb", bufs=3) as out_pool,
            tc.tile_pool(name="pslg", bufs=2, space=bass.MemorySpace.PSUM) as pslg,
            tc.tile_pool(name="psav", bufs=1, space=bass.MemorySpace.PSUM) as psav,
            tc.tile_pool(name="pstr", bufs=1, space=bass.MemorySpace.PSUM) as pstr,
            tc.tile_pool(name="psout", bufs=2, space=bass.MemorySpace.PSUM) as psout,
        ):
            # ---- persistent tiles ----
            xkv_sb = persist.tile([P, KC, S], bf16, tag="xkv")
            kt = persist.tile([P, 4, S], bf16, tag="kt")     # K^T  [feature, s]
            qt = persist.tile([P, 4, S], bf16, tag="qt")     # Q^T
            xt = persist.tile([P, 4, S], bf16, tag="xt")     # attn-out^T, normalized
            # V natural layout [s, head, hd] + denominator column per head
            vsb = persist.tile([P, SC, NH, HD1], bf16, tag="vsb")
            bq_sb = persist.tile([P, 4], f32, tag="bq")
            bk_sb = persist.tile([P, 4], f32, tag="bk")
            bo_sb = persist.tile([P, KC], f32, tag="bo")
            mk_sb = persist.tile([P, SC], f32, tag="mk")
            ident = persist.tile([P, P], bf16, tag="ident")
            vst = persist.tile([P, SC, NH], bf16, tag="vst")

            wk_sb = w_pool.tile([P, KC, F], bf16, tag="wk")
            wv_sb = w_pool.tile([P, KC, F], bf16, tag="wv")
            wq_sb = w_pool.tile([P, KC, F], bf16, tag="wq")
            wo_sb = w_pool.tile([P, 4, D], bf16, tag="wo")

            # c-chunk DMA groups: small first chunks for an early PE start,
            # then bigger ones to amortize the per-DMA HWDGE generation cost.
            GROUPS = [(0, 2), (2, 4), (4, 8)]
            KGROUPS = [(0, 1), (1, 2), (2, 4), (4, 8)]

            for _rep in range(repeat):
                # small side-loads first (ACT queue; tiny transfers), so every
                # consumer below has its dependency edge in program order
                nc.scalar.dma_start(bk_sb[:], bk_d[:].rearrange("(m p) -> p m", p=P))
                nc.scalar.dma_start(bq_sb[:], bq_d[:].rearrange("(m p) -> p m", p=P))
                nc.scalar.dma_start(mk_sb[:], mk_d[:].rearrange("(c p) -> p c", p=P))
                nc.scalar.dma_start(vst[:], vones_d[:])
                nc.scalar.dma_start(ident[:], ident_d[:])
                nc.scalar.dma_start(bo_sb[:], bo_d[:].rearrange("(m p) -> p m", p=P))
                nc.vector.tensor_copy(vsb[:, :, :, HD], vst[:])

                # ---- K / Q projection (feature-major out), c-accumulated ----
                def emit_kq_proj(half, w_d, w_sb, need_w, x_d, p_d, b_sb, dstT,
                                 from_xkv, groups=GROUPS):
                    sl = slice(half * SH, (half + 1) * SH)
                    accA = pslg.tile([P, 2, SH], f32, tag="lg")
                    accB = pslg.tile([P, 2, SH], f32, tag="lg")
                    accs = [accA[:, 0, :], accA[:, 1, :],
                            accB[:, 0, :], accB[:, 1, :]]
                    lin = lin_pool.tile([P, KC, SH], bf16, tag="lin")
                    for g0, g1 in groups:
                        gs = slice(g0 * P, g1 * P)
                        if need_w:
                            nc.sync.dma_start(w_sb[:, g0:g1, :], w_d[gs, :])
                        pr = raw_pool.tile([P, 4, SH], bf16, tag="raw")
                        nc.sync.dma_start(pr[:, 0:g1 - g0, :], p_d[gs, sl])
                        if from_xkv:
                            nc.sync.dma_start(xkv_sb[:, g0:g1, sl], x_d[gs, sl])
                        else:
                            xr = raw_pool.tile([P, 4, SH], bf16, tag="raw")
                            nc.sync.dma_start(xr[:, 0:g1 - g0, :], x_d[gs, sl])
                        for c in range(g0, g1):
                            if from_xkv:
                                nc.vector.tensor_add(
                                    lin[:, c, :], xkv_sb[:, c, sl],
                                    pr[:, c - g0, :])
                            else:
                                nc.vector.tensor_add(
                                    lin[:, c, :], xr[:, c - g0, :],
                                    pr[:, c - g0, :])
                            for m in range(4):
                                nc.tensor.matmul(
                                    accs[m],
                                    w_sb[:, c, m * P:(m + 1) * P],
                                    lin[:, c, :],
                                    start=(c == 0), stop=(c == KC - 1))
                    for m in range(4):
                        nc.vector.tensor_scalar_add(
                            dstT[:, m, sl], accs[m], b_sb[:, m:m + 1])
                    return lin

                emit_kq_proj(0, wk_d, wk_sb, True, xkv_d, pk_d, bk_sb, kt, True)
                emit_kq_proj(1, wk_d, wk_sb, False, xkv_d, pk_d, bk_sb, kt, True)
                qin0 = emit_kq_proj(0, wq_d, wq_sb, True, xq_d, pq_d, bq_sb,
                                    qt, False)

                # ---- V: DMA + adds now; PE chains deferred into the
                # attention stream as fillers (psout ring, 2 live accs) ----
                vins = []
                for half in range(2):
                    sl = slice(half * SH, (half + 1) * SH)
                    vin = lin_pool.tile([P, KC, SH], bf16, tag="lin")
                    for g0, g1 in GROUPS:
                        gs = slice(g0 * P, g1 * P)
                        if half == 0:
                            nc.sync.dma_start(wv_sb[:, g0:g1, :], wv_d[gs, :])
                        pr = raw_pool.tile([P, 4, SH], bf16, tag="raw")
                        nc.sync.dma_start(pr[:, 0:g1 - g0, :], pv_d[gs, sl])
                        for c in range(g0, g1):
                            nc.vector.tensor_add(
                                vin[:, c, :], xkv_sb[:, c, sl],
                                pr[:, c - g0, :])
                    vins.append(vin)

                # ---- Q-half1: DMA + adds now, PE chains deferred ----
                qin1 = lin_pool.tile([P, KC, SH], bf16, tag="lin")
                for g0, g1 in GROUPS:
                    gs = slice(g0 * P, g1 * P)
                    pr = raw_pool.tile([P, 4, SH], bf16, tag="raw")
                    xr = raw_pool.tile([P, 4, SH], bf16, tag="raw")
                    nc.sync.dma_start(pr[:, 0:g1 - g0, :], pq_d[gs, SH:2 * SH])
                    nc.sync.dma_start(xr[:, 0:g1 - g0, :], xq_d[gs, SH:2 * SH])
                    for c in range(g0, g1):
                        nc.vector.tensor_add(
                            qin1[:, c, :], xr[:, c - g0, :], pr[:, c - g0, :])
                nc.sync.dma_start(
                    wo_sb[:], wo_d[:].rearrange("(k p) f -> p k f", p=P))

                # ---- deferred PE chains (fillers) ----
                vdone = [0]

                def emit_vchain(half, sb):
                    vdone[0] += 1
                    acc = psout.tile([P, SH], f32, tag="ps")
                    vin = vins[half]
                    for c in range(KC):
                        nc.tensor.matmul(
                            acc[:],
                            vin[:, c, sb * P:(sb + 1) * P],
                            wv_sb[:, c, :],
                            start=(c == 0), stop=(c == KC - 1))
                    sc = half * 4 + sb
                    nc.vector.tensor_scalar(
                        vsb[:, sc, :, 0:HD],
                        acc[:].rearrange("p (h d) -> p h d", d=HD),
                        mk_sb[:, sc:sc + 1], None,
                        op0=mybir.AluOpType.mult)

                def emit_qh1_chain(m):
                    acc = psout.tile([P, SH], f32, tag="ps")
                    for c in range(KC):
                        nc.tensor.matmul(
                            acc[:],
                            wq_sb[:, c, m * P:(m + 1) * P],
                            qin1[:, c, :],
                            start=(c == 0), stop=(c == KC - 1))
                    nc.vector.tensor_scalar_add(
                        qt[:, m, SH:2 * SH], acc[:], bq_sb[:, m:m + 1])

                def emit_outchain(sh, m):
                    acc = psout.tile([P, SH], f32, tag="ps")
                    for hp in range(4):
                        nc.tensor.matmul(
                            acc[:],
                            wo_sb[:, hp, m * P:(m + 1) * P],
                            xt[:, hp, sh * SH:(sh + 1) * SH],
                            start=(hp == 0), stop=(hp == 3))
                    ob = out_pool.tile([P, SH], f32, tag="outb")
                    nc.vector.tensor_scalar_add(ob[:], acc[:], bo_sb[:, m:m + 1])
                    nc.sync.dma_start(
                        out_d[m * P:(m + 1) * P, sh * SH:(sh + 1) * SH], ob[:])


                # filler queue: (min_slot_index, emit_fn)
                fillers = [
                    (1, lambda: emit_vchain(0, 0)),
                    (2, lambda: emit_vchain(0, 1)),
                    (2, lambda: emit_vchain(0, 2)),
                    (3, lambda: emit_vchain(0, 3)),
                    (3, lambda: emit_vchain(1, 0)),
                    (4, lambda: emit_vchain(1, 1)),
                    (4, lambda: emit_vchain(1, 2)),
                    (5, lambda: emit_vchain(1, 3)),
                    (5, lambda: emit_qh1_chain(0)),
                    (6, lambda: emit_qh1_chain(1)),
                    (6, lambda: emit_qh1_chain(2)),
                    (7, lambda: emit_qh1_chain(3)),
                ]
                # out-h0 chains additionally require every sh=0 AV (heads
                # 0..7) to be emitted, so xt half0 is fully written.
                fillers += [(s, lambda m=m: emit_outchain(0, m), 8)
                            for m, s in enumerate((9, 10, 11, 12, 13, 13, 14, 14))]

                def pop_filler(si):
                    if not fillers or si < fillers[0][0]:
                        return False
                    if len(fillers[0]) > 2 and done_av < fillers[0][2]:
                        return False
                    fillers.pop(0)[1]()
                    return True

                # ---- attention slots, software-pipelined ----
                def emit_qk_pair(sh, h, pt, j):
                    """two s_k chunks (2j, 2j+1) of logits + one exp op."""
                    po = (h % 2) * HD
                    mq = h // 2
                    lg = pslg.tile([P, 2, SH], f32, tag="lg")
                    for i in range(2):
                        c = 2 * j + i
                        nc.tensor.matmul(
                            lg[:, i, :],
                            kt[po:po + HD, mq, c * P:(c + 1) * P],
                            qt[po:po + HD, mq, sh * SH:(sh + 1) * SH],
                            start=True, stop=True)
                    nc.scalar.activation(
                        pt[:, 2 * j:2 * j + 2, :], lg[:],
                        mybir.ActivationFunctionType.Exp)

                def emit_av_mm(h, pt):
                    av = psav.tile([P, 4, P], f32, tag="av")
                    for qb in range(4):
                        for c in range(SC):
                            nc.tensor.matmul(
                                av[:, qb, 0:HD1],
                                pt[:, c, qb * P:(qb + 1) * P],
                                vsb[:, c, h, :],
                                start=(c == 0), stop=(c == SC - 1))
                    return av

                def emit_av_norm(sh, h, av):
                    po = (h % 2) * HD
                    mq = h // 2
                    r = r_pool.tile([P, 4], f32, tag="r")
                    nc.vector.reciprocal(r[:], av[:, :, HD])
                    xn = xn_pool.tile([P, 4, HD], bf16, tag="xn")
                    for qb in range(4):
                        nc.vector.tensor_scalar(
                            xn[:, qb, :], av[:, qb, 0:HD], r[:, qb:qb + 1],
                            None, op0=mybir.AluOpType.mult)
                    xtT = pstr.tile([HD, 4, P], bf16, tag="xtT")
                    for qb in range(4):
                        nc.tensor.transpose(
                            xtT[0:HD, qb, :], xn[:, qb, :], ident[:])
                    nc.vector.tensor_copy(
                        xt[po:po + HD, mq, sh * SH:(sh + 1) * SH],
                        xtT[0:HD, :, :].rearrange("p a b -> p (a b)"))

                def emit_av(sh, h, pt):
                    emit_av_norm(sh, h, emit_av_mm(h, pt))

                # AV work is deferred until V (a filler stream) completes at
                # ~slot 5, then catches up at 2 AVs/slot; the pt ring (7 bufs)
                # covers the lag.
                slots = [(sh, h) for sh in range(2) for h in range(NH)]
                pend = []
                done_av = 0

                def emit_next_av(si):
                    nonlocal done_av
                    if (si >= 6 and vdone[0] == 8 and done_av < len(pend)
                            and done_av < si):
                        emit_av(*pend[done_av])
                        done_av += 1

                for si, (sh, h) in enumerate(slots):
                    pt = pt_pool.tile([P, SC, SH], bf16, tag="pbuf")
                    emit_qk_pair(sh, h, pt, 0)
                    emit_qk_pair(sh, h, pt, 1)
                    emit_next_av(si)
                    pop_filler(si)
                    emit_qk_pair(sh, h, pt, 2)
                    emit_qk_pair(sh, h, pt, 3)
                    if not pop_filler(si):
                        emit_next_av(si)
                    pend.append((sh, h, pt))
                while done_av < len(pend):
                    emit_av(*pend[done_av])
                    done_av += 1
                while fillers:
                    fillers.pop(0)[1]()
                for m in range(KC):
                    emit_outchain(1, m)

    nc.compile()
    return nc


_program = None
_last_in_maps = None


def _get_program():
    global _program
    if _program is None:
        _program = build_program()
    return _program


def kernel(inputs_q, inputs_kv, pos_emb_q, pos_emb_k, pos_emb_v,
           key_padding_mask, wq, bq, wk, bk, wv, bv, wo, bo):
    nc = _get_program()

    bf = mybir.dt.np(bf16)

    wqf = np.asarray(wq, np.float32).reshape(D, H * HD)
    wkf = np.asarray(wk, np.float32).reshape(D, H * HD)
    wvf = np.asarray(wv, np.float32).reshape(D, H * HD)
    wof = np.asarray(wo, np.float32).reshape(H * HD, D)
    bqf = np.asarray(bq, np.float32).reshape(H * HD)
    bkf = np.asarray(bk, np.float32).reshape(H * HD)
    bvf = np.asarray(bv, np.float32).reshape(H * HD)
    bof = np.asarray(bo, np.float32).reshape(D)
    # bv is structurally zero in this problem; it has no cheap slot in the
    # transposed dataflow, so refuse loudly rather than silently drop it.
    assert np.all(bvf == 0.0), "nonzero bv is not supported"

    scale = np.float32(1.0 / np.sqrt(HD))
    iq = np.asarray(inputs_q, np.float32)
    ikv = np.asarray(inputs_kv, np.float32)
    pqa = np.asarray(pos_emb_q, np.float32)
    pka = np.asarray(pos_emb_k, np.float32)
    pva = np.asarray(pos_emb_v, np.float32)
    mask = np.asarray(key_padding_mask, np.float32)

    ident_np = np.eye(P, dtype=bf)

    in_maps = []
    for b in range(B):
        xq_t = np.ascontiguousarray(iq[b].T.astype(bf))
        xkv_t = np.ascontiguousarray(ikv[b].T.astype(bf))
        pq_t = np.ascontiguousarray(pqa[b].T.astype(bf))
        pk_t = np.ascontiguousarray(pka[b].T.astype(bf))
        pv_t = np.ascontiguousarray(pva[b].T.astype(bf))
        mk = np.ascontiguousarray(mask[b])
        # mask value per (partition, s-chunk, head) for V's denom column
        vones = np.ascontiguousarray(
            np.broadcast_to(mk.reshape(SC, P).T[:, :, None], (P, SC, NH))
        ).astype(bf)
        for hg in range(2):
            sl = slice(hg * F, (hg + 1) * F)
            in_maps.append({
                "xq": xq_t, "xkv": xkv_t, "pq": pq_t, "pk": pk_t, "pv": pv_t,
                "wq": np.ascontiguousarray(wqf[:, sl] * scale).astype(bf),
                "wk": np.ascontiguousarray(wkf[:, sl]).astype(bf),
                "wv": np.ascontiguousarray(wvf[:, sl]).astype(bf),
                "wo": np.ascontiguousarray(wof[sl, :]).astype(bf),
                "bq": np.ascontiguousarray(bqf[sl]) * scale,
                "bk": np.ascontiguousarray(bkf[sl]),
                "bo": bof if hg == 0 else np.zeros_like(bof),
                "mk": mk,
                "vones": vones,
                "ident": ident_np,
            })

    global _last_in_maps
    _last_in_maps = in_maps
    res = run_bass_kernel_spmd(nc, in_maps, list(range(2 * B)))
    outs = [res.results[i]["out_t"] for i in range(2 * B)]
    out = np.stack([(outs[2 * b] + outs[2 * b + 1]).T for b in range(B)])
    return np.ascontiguousarray(out, dtype=np.float32)


# revision 40
# speedup vs baseline: 1.0116x; 1.0116x over previous
# DETR multi-head dot-product attention for Trainium2 (Bass/Tile), 8 NeuronCores.
#
# Problem (hardcoded): B=4, S=1024, D=1024, H=16, HD=64, f32.
#   q = (inputs_q + pos_emb_q) @ wq + bq;  q /= sqrt(HD)
#   k = (inputs_kv + pos_emb_k) @ wk + bk
#   v = (inputs_kv + pos_emb_v) @ wv + bv          (bv == 0 by problem spec)
#   attn = softmax(q k^T + key_padding_bias); out = (attn v) @ wo + bo
#
# Sharding: 8 cores = 4 batches x 2 head-groups of 8 heads. Each core computes
# its batch's projections restricted to its head-group's features (512 of 1024),
# full attention for its 8 heads, and a partial output projection. The host
# sums the two head-group partials per batch.
#
# All matmul operands are bf16 (f32 PSUM accumulation); rel-err budget (2e-2)
# has ~10x headroom over bf16 noise. bf16 removes the f32r small-N penalty,
# which unlocks the cheap AV orientation: lhsT = P^T chunk [s_k,128 s_q],
# rhs = V chunk [s_k, HD+1] -> av[s_q, HD+1] at 65 cycles per chunk matmul
# (vs 512 the other way). Column HD of V is the padding mask, so av[:, HD]
# is the softmax denominator as a per-partition scalar: normalization is a
# reciprocal + per-partition tensor_scalar multiply, no broadcast matmul.
# A bf16 PE transpose restores feature-major x^T for the output projection;
# the Pool engine copies it PSUM->SBUF to keep DVE/ACT free.
#
# Schedule: projections accumulate over feature chunks in PSUM so the PE
# starts ~1.5us in, as soon as the first weight/input chunks land (the
# previous design waited ~22us for whole tensors). Phase order K, V, Q keeps
# the DMA queue aligned with consumption order; attention is software-
# pipelined across (s_q-half, head) slots with exp (ACT) as the pacing
# engine, and the Q-half1 projection + output projections are interleaved
# into the slot stream as PE filler.

import sys

for _p in ("/opt/trn_rl_repo", "/root/.axon_site/_ro/trn_rl_repo"):
    if _p not in sys.path:
        sys.path.append(_p)

import numpy as np

import concourse.bass as bass
import concourse.mybir as mybir
import concourse.tile as tile
from concourse import bacc
from concourse.bass_utils import run_bass_kernel_spmd

B, S, D = 4, 1024, 1024
H, HD = 16, 64
F = 512          # features per head-group core (8 heads * 64)
NH = 8           # heads per core
P = 128          # partitions
KC = D // P      # contraction chunks for the input projections (8)
SC = S // P      # sequence chunks (8)
SH = 512         # S-half
HD1 = HD + 1     # head_dim + denominator column

f32 = mybir.dt.float32
bf16 = mybir.dt.bfloat16


def build_program(repeat=1):
    nc = bacc.Bacc("TRN2", target_bir_lowering=False, debug=False)

    xq_d = nc.dram_tensor("xq", [D, S], bf16, kind="ExternalInput")
    xkv_d = nc.dram_tensor("xkv", [D, S], bf16, kind="ExternalInput")
    pq_d = nc.dram_tensor("pq", [D, S], bf16, kind="ExternalInput")
    pk_d = nc.dram_tensor("pk", [D, S], bf16, kind="ExternalInput")
    pv_d = nc.dram_tensor("pv", [D, S], bf16, kind="ExternalInput")
    wq_d = nc.dram_tensor("wq", [D, F], bf16, kind="ExternalInput")
    wk_d = nc.dram_tensor("wk", [D, F], bf16, kind="ExternalInput")
    wv_d = nc.dram_tensor("wv", [D, F], bf16, kind="ExternalInput")
    wo_d = nc.dram_tensor("wo", [F, D], bf16, kind="ExternalInput")
    bq_d = nc.dram_tensor("bq", [F], f32, kind="ExternalInput")
    bk_d = nc.dram_tensor("bk", [F], f32, kind="ExternalInput")
    bo_d = nc.dram_tensor("bo", [D], f32, kind="ExternalInput")
    mk_d = nc.dram_tensor("mk", [S], f32, kind="ExternalInput")  # padding mask
    # mask replicated per head for V's denominator column
    vones_d = nc.dram_tensor("vones", [P, SC, NH], bf16, kind="ExternalInput")
    ident_d = nc.dram_tensor("ident", [P, P], bf16, kind="ExternalInput")
    out_d = nc.dram_tensor("out_t", [D, S], f32, kind="ExternalOutput")

    with tile.TileContext(nc) as tc:
        with (
            tc.tile_pool(name="persist", bufs=1) as persist,
            tc.tile_pool(name="wmat", bufs=1) as w_pool,
            tc.tile_pool(name="lin", bufs=3) as lin_pool,
            tc.tile_pool(name="raw", bufs=5) as raw_pool,
            tc.tile_pool(name="ptp", bufs=7) as pt_pool,
            tc.tile_pool(name="xnp", bufs=2) as xn_pool,
            tc.tile_pool(name="rp", bufs=2) as r_pool,
            tc.tile_pool(name="outb", bufs=3) as out_pool,
            tc.tile_pool(name="outb4", bufs=2) as out4_pool,
            tc.tile_pool(name="pslg", bufs=2, space=bass.MemorySpace.PSUM) as pslg,
            tc.tile_pool(name="psav", bufs=1, space=bass.MemorySpace.PSUM) as psav,
            tc.tile_pool(name="pstr", bufs=1, space=bass.MemorySpace.PSUM) as pstr,
            tc.tile_pool(name="psout", bufs=2, space=bass.MemorySpace.PSUM) as psout,
        ):
            # ---- persistent tiles ----
            xkv_sb = persist.tile([P, KC, S], bf16, tag="xkv")
            kt = persist.tile([P, 4, S], bf16, tag="kt")     # K^T  [feature, s]
            qt = persist.tile([P, 4, S], bf16, tag="qt")     # Q^T
            xt = persist.tile([P, 4, S], bf16, tag="xt")     # attn-out^T, normalized
            # V natural layout [s, head, hd] + denominator column per head
            vsb = persist.tile([P, SC, NH, HD1], bf16, tag="vsb")
            bq_sb = persist.tile([P, 4], f32, tag="bq")
            bk_sb = persist.tile([P, 4], f32, tag="bk")
            bo_sb = persist.tile([P, KC], f32, tag="bo")
            mk_sb = persist.tile([P, SC], f32, tag="mk")
            ident = persist.tile([P, P], bf16, tag="ident")
            vst = persist.tile([P, SC, NH], bf16, tag="vst")

            wk_sb = w_pool.tile([P, KC, F], bf16, tag="wk")
            wv_sb = w_pool.tile([P, KC, F], bf16, tag="wv")
            wq_sb = w_pool.tile([P, KC, F], bf16, tag="wq")
            wo_sb = w_pool.tile([P, 4, D], bf16, tag="wo")

            # c-chunk DMA groups: small first chunks for an early PE start,
            # then bigger ones to amortize the per-DMA HWDGE generation cost.
            GROUPS = [(0, 2), (2, 4), (4, 8)]
            KGROUPS = [(0, 1), (1, 2), (2, 4), (4, 8)]

            for _rep in range(repeat):
                # small side-loads first (ACT queue; tiny transfers), so every
                # consumer below has its dependency edge in program order
                nc.scalar.dma_start(bk_sb[:], bk_d[:].rearrange("(m p) -> p m", p=P))
                nc.scalar.dma_start(bq_sb[:], bq_d[:].rearrange("(m p) -> p m", p=P))
                nc.scalar.dma_start(mk_sb[:], mk_d[:].rearrange("(c p) -> p c", p=P))
                nc.scalar.dma_start(vst[:], vones_d[:])
                nc.scalar.dma_start(ident[:], ident_d[:])
                nc.scalar.dma_start(bo_sb[:], bo_d[:].rearrange("(m p) -> p m", p=P))
                nc.vector.tensor_copy(vsb[:, :, :, HD], vst[:])

                # ---- K / Q projection (feature-major out), c-accumulated ----
                def emit_kq_proj(half, w_d, w_sb, need_w, x_d, p_d, b_sb, dstT,
                                 from_xkv, groups=GROUPS):
                    sl = slice(half * SH, (half + 1) * SH)
                    accA = pslg.tile([P, 2, SH], f32, tag="lg")
                    accB = pslg.tile([P, 2, SH], f32, tag="lg")
                    accs = [accA[:, 0, :], accA[:, 1, :],
                            accB[:, 0, :], accB[:, 1, :]]
                    lin = lin_pool.tile([P, KC, SH], bf16, tag="lin")
                    for g0, g1 in groups:
                        gs = slice(g0 * P, g1 * P)
                        if need_w:
                            nc.sync.dma_start(w_sb[:, g0:g1, :], w_d[gs, :])
                        pr = raw_pool.tile([P, 4, SH], bf16, tag="raw")
                        nc.sync.dma_start(pr[:, 0:g1 - g0, :], p_d[gs, sl])
                        if from_xkv:
                            nc.sync.dma_start(xkv_sb[:, g0:g1, sl], x_d[gs, sl])
                        else:
                            xr = raw_pool.tile([P, 4, SH], bf16, tag="raw")
                            nc.sync.dma_start(xr[:, 0:g1 - g0, :], x_d[gs, sl])
                        for c in range(g0, g1):
                            if from_xkv:
                                nc.vector.tensor_add(
                                    lin[:, c, :], xkv_sb[:, c, sl],
                                    pr[:, c - g0, :])
                            else:
                                nc.vector.tensor_add(
                                    lin[:, c, :], xr[:, c - g0, :],
                                    pr[:, c - g0, :])
                            for m in range(4):
                                nc.tensor.matmul(
                                    accs[m],
                                    w_sb[:, c, m * P:(m + 1) * P],
                                    lin[:, c, :],
                                    start=(c == 0), stop=(c == KC - 1))
                    for m in range(4):
                        nc.vector.tensor_scalar_add(
                            dstT[:, m, sl], accs[m], b_sb[:, m:m + 1])
                    return lin

                emit_kq_proj(0, wk_d, wk_sb, True, xkv_d, pk_d, bk_sb, kt, True)
                emit_kq_proj(1, wk_d, wk_sb, False, xkv_d, pk_d, bk_sb, kt, True)
                qin0 = emit_kq_proj(0, wq_d, wq_sb, True, xq_d, pq_d, bq_sb,
                                    qt, False)

                # ---- V: DMA + adds now; PE chains deferred into the
                # attention stream as fillers (psout ring, 2 live accs) ----
                vins = []
                for half in range(2):
                    sl = slice(half * SH, (half + 1) * SH)
                    vin = lin_pool.tile([P, KC, SH], bf16, tag="lin")
                    for g0, g1 in GROUPS:
                        gs = slice(g0 * P, g1 * P)
                        if half == 0:
                            nc.sync.dma_start(wv_sb[:, g0:g1, :], wv_d[gs, :])
                        pr = raw_pool.tile([P, 4, SH], bf16, tag="raw")
                        nc.sync.dma_start(pr[:, 0:g1 - g0, :], pv_d[gs, sl])
                        for c in range(g0, g1):
                            nc.vector.tensor_add(
                                vin[:, c, :], xkv_sb[:, c, sl],
                                pr[:, c - g0, :])
                    vins.append(vin)

                # ---- Q-half1: DMA + adds now, PE chains deferred ----
                qin1 = lin_pool.tile([P, KC, SH], bf16, tag="lin")
                for g0, g1 in GROUPS:
                    gs = slice(g0 * P, g1 * P)
                    pr = raw_pool.tile([P, 4, SH], bf16, tag="raw")
                    xr = raw_pool.tile([P, 4, SH], bf16, tag="raw")
                    nc.sync.dma_start(pr[:, 0:g1 - g0, :], pq_d[gs, SH:2 * SH])
                    nc.sync.dma_start(xr[:, 0:g1 - g0, :], xq_d[gs, SH:2 * SH])
                    for c in range(g0, g1):
                        nc.vector.tensor_add(
                            qin1[:, c, :], xr[:, c - g0, :], pr[:, c - g0, :])
                nc.sync.dma_start(
                    wo_sb[:], wo_d[:].rearrange("(k p) f -> p k f", p=P))

                # ---- deferred PE chains (fillers) ----
                vdone = [0]

                def emit_vchain(half, sb):
                    vdone[0] += 1
                    acc = psout.tile([P, SH], f32, tag="ps")
                    vin = vins[half]
                    for c in range(KC):
                        nc.tensor.matmul(
                            acc[:],
                            vin[:, c, sb * P:(sb + 1) * P],
                            wv_sb[:, c, :],
                            start=(c == 0), stop=(c == KC - 1))
                    sc = half * 4 + sb
                    nc.vector.tensor_scalar(
                        vsb[:, sc, :, 0:HD],
                        acc[:].rearrange("p (h d) -> p h d", d=HD),
                        mk_sb[:, sc:sc + 1], None,
                        op0=mybir.AluOpType.mult)

                def emit_qh1_chain(m):
                    acc = psout.tile([P, SH], f32, tag="ps")
                    for c in range(KC):
                        nc.tensor.matmul(
                            acc[:],
                            wq_sb[:, c, m * P:(m + 1) * P],
                            qin1[:, c, :],
                            start=(c == 0), stop=(c == KC - 1))
                    nc.vector.tensor_scalar_add(
                        qt[:, m, SH:2 * SH], acc[:], bq_sb[:, m:m + 1])

                def emit_outchain(sh, m):
                    acc = psout.tile([P, SH], f32, tag="ps")
                    for hp in range(4):
                        nc.tensor.matmul(
                            acc[:],
                            wo_sb[:, hp, m * P:(m + 1) * P],
                            xt[:, hp, sh * SH:(sh + 1) * SH],
                            start=(hp == 0), stop=(hp == 3))
                    ob = out_pool.tile([P, SH], f32, tag="outb")
                    nc.vector.tensor_scalar_add(ob[:], acc[:], bo_sb[:, m:m + 1])
                    nc.sync.dma_start(
                        out_d[m * P:(m + 1) * P, sh * SH:(sh + 1) * SH], ob[:])


                # filler queue: (min_slot_index, emit_fn)
                fillers = [
                    (1, lambda: emit_vchain(0, 0)),
                    (2, lambda: emit_vchain(0, 1)),
                    (2, lambda: emit_vchain(0, 2)),
                    (3, lambda: emit_vchain(0, 3)),
                    (3, lambda: emit_vchain(1, 0)),
                    (4, lambda: emit_vchain(1, 1)),
                    (4, lambda: emit_vchain(1, 2)),
                    (5, lambda: emit_vchain(1, 3)),
                    (5, lambda: emit_qh1_chain(0)),
                    (6, lambda: emit_qh1_chain(1)),
                    (6, lambda: emit_qh1_chain(2)),
                    (7, lambda: emit_qh1_chain(3)),
                ]
                # out-h0 chains additionally require every sh=0 AV (heads
                # 0..7) to be emitted, so xt half0 is fully written.
                fillers += [(s, lambda m=m: emit_outchain(0, m), 8)
                            for m, s in enumerate((9, 10, 11, 12, 13, 13, 14, 14))]

                def pop_filler(si):
                    if not fillers or si < fillers[0][0]:
                        return False
                    if len(fillers[0]) > 2 and done_av < fillers[0][2]:
                        return False
                    fillers.pop(0)[1]()
                    return True

                # ---- attention slots, software-pipelined ----
                def emit_qk_pair(sh, h, pt, j):
                    """two s_k chunks (2j, 2j+1) of logits + one exp op."""
                    po = (h % 2) * HD
                    mq = h // 2
                    lg = pslg.tile([P, 2, SH], f32, tag="lg")
                    for i in range(2):
                        c = 2 * j + i
                        nc.tensor.matmul(
                            lg[:, i, :],
                            kt[po:po + HD, mq, c * P:(c + 1) * P],
                            qt[po:po + HD, mq, sh * SH:(sh + 1) * SH],
                            start=True, stop=True)
                    nc.scalar.activation(
                        pt[:, 2 * j:2 * j + 2, :], lg[:],
                        mybir.ActivationFunctionType.Exp)

                def emit_av_mm(h, pt):
                    av = psav.tile([P, 4, P], f32, tag="av")
                    for qb in range(4):
                        for c in range(SC):
                            nc.tensor.matmul(
                                av[:, qb, 0:HD1],
                                pt[:, c, qb * P:(qb + 1) * P],
                                vsb[:, c, h, :],
                                start=(c == 0), stop=(c == SC - 1))
                    return av

                def emit_av_norm(sh, h, av):
                    po = (h % 2) * HD
                    mq = h // 2
                    r = r_pool.tile([P, 4], f32, tag="r")
                    nc.vector.reciprocal(r[:], av[:, :, HD])
                    xn = xn_pool.tile([P, 4, HD], bf16, tag="xn")
                    for qb in range(4):
                        nc.vector.tensor_scalar(
                            xn[:, qb, :], av[:, qb, 0:HD], r[:, qb:qb + 1],
                            None, op0=mybir.AluOpType.mult)
                    xtT = pstr.tile([HD, 4, P], bf16, tag="xtT")
                    for qb in range(4):
                        nc.tensor.transpose(
                            xtT[0:HD, qb, :], xn[:, qb, :], ident[:])
                    nc.vector.tensor_copy(
                        xt[po:po + HD, mq, sh * SH:(sh + 1) * SH],
                        xtT[0:HD, :, :].rearrange("p a b -> p (a b)"))

                def emit_av(sh, h, pt):
                    emit_av_norm(sh, h, emit_av_mm(h, pt))

                # AV work is deferred until V (a filler stream) completes at
                # ~slot 5, then catches up at 2 AVs/slot; the pt ring (7 bufs)
                # covers the lag.
                slots = [(sh, h) for sh in range(2) for h in range(NH)]
                pend = []
                done_av = 0

                def emit_next_av(si):
                    nonlocal done_av
                    if (si >= 6 and vdone[0] == 8 and done_av < len(pend)
                            and done_av < si):
                        emit_av(*pend[done_av])
                        done_av += 1

                for si, (sh, h) in enumerate(slots):
                    pt = pt_pool.tile([P, SC, SH], bf16, tag="pbuf")
                    emit_qk_pair(sh, h, pt, 0)
                    emit_qk_pair(sh, h, pt, 1)
                    emit_next_av(si)
                    pop_filler(si)
                    emit_qk_pair(sh, h, pt, 2)
                    emit_qk_pair(sh, h, pt, 3)
                    if not pop_filler(si):
                        emit_next_av(si)
                    pend.append((sh, h, pt))
                while done_av < len(pend):
                    emit_av(*pend[done_av])
                    done_av += 1
                while fillers:
                    fillers.pop(0)[1]()
                # half-1 out-projection: plain chains, but biases land in
                # 4-chunk buffers so the tail DMAs go out 2 chunks at a time
                # (half the HWDGE generation serialization)
                ob4a = out4_pool.tile([P, 4, SH], f32, tag="outb4")
                ob4b = out4_pool.tile([P, 4, SH], f32, tag="outb4")
                ob4 = [ob4a, ob4b]
                for m in range(KC):
                    acc = psout.tile([P, SH], f32, tag="ps")
                    for hp in range(4):
                        nc.tensor.matmul(
                            acc[:],
                            wo_sb[:, hp, m * P:(m + 1) * P],
                            xt[:, hp, SH:2 * SH],
                            start=(hp == 0), stop=(hp == 3))
                    nc.vector.tensor_scalar_add(
                        ob4[m // 4][:, m % 4, :], acc[:], bo_sb[:, m:m + 1])
                    if m % 2 == 1:
                        nc.sync.dma_start(
                            out_d[(m - 1) * P:(m + 1) * P, SH:2 * SH]
                            .rearrange("(j p) s -> p j s", p=P),
                            ob4[m // 4][:, m % 4 - 1:m % 4 + 1, :])

    nc.compile()
    return nc


_program = None
_last_in_maps = None


def _get_program():
    global _program
    if _program is None:
        _program = build_program()
    return _program


def kernel(inputs_q, inputs_kv, pos_emb_q, pos_emb_k, pos_emb_v,
           key_padding_mask, wq, bq, wk, bk, wv, bv, wo, bo):
    nc = _get_program()

    bf = mybir.dt.np(bf16)

    wqf = np.asarray(wq, np.float32).reshape(D, H * HD)
    wkf = np.asarray(wk, np.float32).reshape(D, H * HD)
    wvf = np.asarray(wv, np.float32).reshape(D, H * HD)
    wof = np.asarray(wo, np.float32).reshape(H * HD, D)
    bqf = np.asarray(bq, np.float32).reshape(H * HD)
    bkf = np.asarray(bk, np.float32).reshape(H * HD)
    bvf = np.asarray(bv, np.float32).reshape(H * HD)
    bof = np.asarray(bo, np.float32).reshape(D)
    # bv is structurally zero in this problem; it has no cheap slot in the
    # transposed dataflow, so refuse loudly rather than silently drop it.
    assert np.all(bvf == 0.0), "nonzero bv is not supported"

    scale = np.float32(1.0 / np.sqrt(HD))
    iq = np.asarray(inputs_q, np.float32)
    ikv = np.asarray(inputs_kv, np.float32)
    pqa = np.asarray(pos_emb_q, np.float32)
    pka = np.asarray(pos_emb_k, np.float32)
    pva = np.asarray(pos_emb_v, np.float32)
    mask = np.asarray(key_padding_mask, np.float32)

    ident_np = np.eye(P, dtype=bf)

    in_maps = []
    for b in range(B):
        xq_t = np.ascontiguousarray(iq[b].T.astype(bf))
        xkv_t = np.ascontiguousarray(ikv[b].T.astype(bf))
        pq_t = np.ascontiguousarray(pqa[b].T.astype(bf))
        pk_t = np.ascontiguousarray(pka[b].T.astype(bf))
        pv_t = np.ascontiguousarray(pva[b].T.astype(bf))
        mk = np.ascontiguousarray(mask[b])
        # mask value per (partition, s-chunk, head) for V's denom column
        vones = np.ascontiguousarray(
            np.broadcast_to(mk.reshape(SC, P).T[:, :, None], (P, SC, NH))
        ).astype(bf)
        for hg in range(2):
            sl = slice(hg * F, (hg + 1) * F)
            in_maps.append({
                "xq": xq_t, "xkv": xkv_t, "pq": pq_t, "pk": pk_t, "pv": pv_t,
                "wq": np.ascontiguousarray(wqf[:, sl] * scale).astype(bf),
                "wk": np.ascontiguousarray(wkf[:, sl]).astype(bf),
                "wv": np.ascontiguousarray(wvf[:, sl]).astype(bf),
                "wo": np.ascontiguousarray(wof[sl, :]).astype(bf),
                "bq": np.ascontiguousarray(bqf[sl]) * scale,
                "bk": np.ascontiguousarray(bkf[sl]),
                "bo": bof if hg == 0 else np.zeros_like(bof),
                "mk": mk,
                "vones": vones,
                "ident": ident_np,
            })

    global _last_in_maps
    _last_in_maps = in_maps
    res = run_bass_kernel_spmd(nc, in_maps, list(range(2 * B)))
    outs = [res.results[i]["out_t"] for i in range(2 * B)]
    out = np.stack([(outs[2 * b] + outs[2 * b + 1]).T for b in range(B)])
    return np.ascontiguousarray(out, dtype=np.float32)


# revision 41
# speedup vs baseline: 1.0185x; 1.0068x over previous
# DETR multi-head dot-product attention for Trainium2 (Bass/Tile), 8 NeuronCores.
#
# Problem (hardcoded): B=4, S=1024, D=1024, H=16, HD=64, f32.
#   q = (inputs_q + pos_emb_q) @ wq + bq;  q /= sqrt(HD)
#   k = (inputs_kv + pos_emb_k) @ wk + bk
#   v = (inputs_kv + pos_emb_v) @ wv + bv          (bv == 0 by problem spec)
#   attn = softmax(q k^T + key_padding_bias); out = (attn v) @ wo + bo
#
# Sharding: 8 cores = 4 batches x 2 head-groups of 8 heads. Each core computes
# its batch's projections restricted to its head-group's features (512 of 1024),
# full attention for its 8 heads, and a partial output projection. The host
# sums the two head-group partials per batch.
#
# All matmul operands are bf16 (f32 PSUM accumulation); rel-err budget (2e-2)
# has ~10x headroom over bf16 noise. bf16 removes the f32r small-N penalty,
# which unlocks the cheap AV orientation: lhsT = P^T chunk [s_k,128 s_q],
# rhs = V chunk [s_k, HD+1] -> av[s_q, HD+1] at 65 cycles per chunk matmul
# (vs 512 the other way). Column HD of V is the padding mask, so av[:, HD]
# is the softmax denominator as a per-partition scalar: normalization is a
# reciprocal + per-partition tensor_scalar multiply, no broadcast matmul.
# A bf16 PE transpose restores feature-major x^T for the output projection;
# the Pool engine copies it PSUM->SBUF to keep DVE/ACT free.
#
# Schedule: projections accumulate over feature chunks in PSUM so the PE
# starts ~1.5us in, as soon as the first weight/input chunks land (the
# previous design waited ~22us for whole tensors). Phase order K, V, Q keeps
# the DMA queue aligned with consumption order; attention is software-
# pipelined across (s_q-half, head) slots with exp (ACT) as the pacing
# engine, and the Q-half1 projection + output projections are interleaved
# into the slot stream as PE filler.

import sys

for _p in ("/opt/trn_rl_repo", "/root/.axon_site/_ro/trn_rl_repo"):
    if _p not in sys.path:
        sys.path.append(_p)

import numpy as np

import concourse.bass as bass
import concourse.mybir as mybir
import concourse.tile as tile
from concourse import bacc
from concourse.bass_utils import run_bass_kernel_spmd

B, S, D = 4, 1024, 1024
H, HD = 16, 64
F = 512          # features per head-group core (8 heads * 64)
NH = 8           # heads per core
P = 128          # partitions
KC = D // P      # contraction chunks for the input projections (8)
SC = S // P      # sequence chunks (8)
SH = 512         # S-half
HD1 = HD + 1     # head_dim + denominator column

f32 = mybir.dt.float32
bf16 = mybir.dt.bfloat16


def build_program(repeat=1):
    nc = bacc.Bacc("TRN2", target_bir_lowering=False, debug=False)

    xq_d = nc.dram_tensor("xq", [D, S], bf16, kind="ExternalInput")
    xkv_d = nc.dram_tensor("xkv", [D, S], bf16, kind="ExternalInput")
    pq_d = nc.dram_tensor("pq", [D, S], bf16, kind="ExternalInput")
    pk_d = nc.dram_tensor("pk", [D, S], bf16, kind="ExternalInput")
    pv_d = nc.dram_tensor("pv", [D, S], bf16, kind="ExternalInput")
    wq_d = nc.dram_tensor("wq", [D, F], bf16, kind="ExternalInput")
    wk_d = nc.dram_tensor("wk", [D, F], bf16, kind="ExternalInput")
    wv_d = nc.dram_tensor("wv", [D, F], bf16, kind="ExternalInput")
    wo_d = nc.dram_tensor("wo", [F, D], bf16, kind="ExternalInput")
    bq_d = nc.dram_tensor("bq", [F], f32, kind="ExternalInput")
    bk_d = nc.dram_tensor("bk", [F], f32, kind="ExternalInput")
    bo_d = nc.dram_tensor("bo", [D], f32, kind="ExternalInput")
    mk_d = nc.dram_tensor("mk", [S], f32, kind="ExternalInput")  # padding mask
    # mask replicated per head for V's denominator column
    vones_d = nc.dram_tensor("vones", [P, SC, NH], bf16, kind="ExternalInput")
    ident_d = nc.dram_tensor("ident", [P, P], bf16, kind="ExternalInput")
    out_d = nc.dram_tensor("out_t", [D, S], f32, kind="ExternalOutput")

    with tile.TileContext(nc) as tc:
        with (
            tc.tile_pool(name="persist", bufs=1) as persist,
            tc.tile_pool(name="wmat", bufs=1) as w_pool,
            tc.tile_pool(name="lin", bufs=3) as lin_pool,
            tc.tile_pool(name="raw", bufs=5) as raw_pool,
            tc.tile_pool(name="ptp", bufs=7) as pt_pool,
            tc.tile_pool(name="xnp", bufs=2) as xn_pool,
            tc.tile_pool(name="rp", bufs=2) as r_pool,
            tc.tile_pool(name="outb", bufs=3) as out_pool,
            tc.tile_pool(name="outb4", bufs=2) as out4_pool,
            tc.tile_pool(name="pslg", bufs=2, space=bass.MemorySpace.PSUM) as pslg,
            tc.tile_pool(name="psav", bufs=1, space=bass.MemorySpace.PSUM) as psav,
            tc.tile_pool(name="pstr", bufs=1, space=bass.MemorySpace.PSUM) as pstr,
            tc.tile_pool(name="psout", bufs=2, space=bass.MemorySpace.PSUM) as psout,
        ):
            # ---- persistent tiles ----
            xkv_sb = persist.tile([P, KC, S], bf16, tag="xkv")
            kt = persist.tile([P, 4, S], bf16, tag="kt")     # K^T  [feature, s]
            qt = persist.tile([P, 4, S], bf16, tag="qt")     # Q^T
            xt = persist.tile([P, 4, S], bf16, tag="xt")     # attn-out^T, normalized
            # V natural layout [s, head, hd] + denominator column per head
            vsb = persist.tile([P, SC, NH, HD1], bf16, tag="vsb")
            bq_sb = persist.tile([P, 4], f32, tag="bq")
            bk_sb = persist.tile([P, 4], f32, tag="bk")
            bo_sb = persist.tile([P, KC], f32, tag="bo")
            mk_sb = persist.tile([P, SC], f32, tag="mk")
            ident = persist.tile([P, P], bf16, tag="ident")
            vst = persist.tile([P, SC, NH], bf16, tag="vst")

            wk_sb = w_pool.tile([P, KC, F], bf16, tag="wk")
            wv_sb = w_pool.tile([P, KC, F], bf16, tag="wv")
            wq_sb = w_pool.tile([P, KC, F], bf16, tag="wq")
            wo_sb = w_pool.tile([P, 4, D], bf16, tag="wo")

            # c-chunk DMA groups: small first chunks for an early PE start,
            # then bigger ones to amortize the per-DMA HWDGE generation cost.
            GROUPS = [(0, 2), (2, 4), (4, 8)]
            KGROUPS = [(0, 1), (1, 2), (2, 4), (4, 8)]

            for _rep in range(repeat):
                # small side-loads first (ACT queue; tiny transfers), so every
                # consumer below has its dependency edge in program order
                nc.scalar.dma_start(bk_sb[:], bk_d[:].rearrange("(m p) -> p m", p=P))
                nc.scalar.dma_start(bq_sb[:], bq_d[:].rearrange("(m p) -> p m", p=P))
                nc.scalar.dma_start(mk_sb[:], mk_d[:].rearrange("(c p) -> p c", p=P))
                nc.scalar.dma_start(vst[:], vones_d[:])
                nc.scalar.dma_start(ident[:], ident_d[:])
                nc.scalar.dma_start(bo_sb[:], bo_d[:].rearrange("(m p) -> p m", p=P))
                nc.vector.tensor_copy(vsb[:, :, :, HD], vst[:])

                # ---- K / Q projection (feature-major out), c-accumulated ----
                def emit_kq_proj(half, w_d, w_sb, need_w, x_d, p_d, b_sb, dstT,
                                 from_xkv, groups=GROUPS):
                    sl = slice(half * SH, (half + 1) * SH)
                    accA = pslg.tile([P, 2, SH], f32, tag="lg")
                    accB = pslg.tile([P, 2, SH], f32, tag="lg")
                    accs = [accA[:, 0, :], accA[:, 1, :],
                            accB[:, 0, :], accB[:, 1, :]]
                    lin = lin_pool.tile([P, KC, SH], bf16, tag="lin")
                    for g0, g1 in groups:
                        gs = slice(g0 * P, g1 * P)
                        if need_w:
                            nc.sync.dma_start(w_sb[:, g0:g1, :], w_d[gs, :])
                        pr = raw_pool.tile([P, 4, SH], bf16, tag="raw")
                        nc.sync.dma_start(pr[:, 0:g1 - g0, :], p_d[gs, sl])
                        if from_xkv:
                            nc.sync.dma_start(xkv_sb[:, g0:g1, sl], x_d[gs, sl])
                        else:
                            xr = raw_pool.tile([P, 4, SH], bf16, tag="raw")
                            nc.sync.dma_start(xr[:, 0:g1 - g0, :], x_d[gs, sl])
                        for c in range(g0, g1):
                            if from_xkv:
                                nc.vector.tensor_add(
                                    lin[:, c, :], xkv_sb[:, c, sl],
                                    pr[:, c - g0, :])
                            else:
                                nc.vector.tensor_add(
                                    lin[:, c, :], xr[:, c - g0, :],
                                    pr[:, c - g0, :])
                            for m in range(4):
                                nc.tensor.matmul(
                                    accs[m],
                                    w_sb[:, c, m * P:(m + 1) * P],
                                    lin[:, c, :],
                                    start=(c == 0), stop=(c == KC - 1))
                    for m in range(4):
                        nc.vector.tensor_scalar_add(
                            dstT[:, m, sl], accs[m], b_sb[:, m:m + 1])
                    return lin

                emit_kq_proj(0, wk_d, wk_sb, True, xkv_d, pk_d, bk_sb, kt, True)
                emit_kq_proj(1, wk_d, wk_sb, False, xkv_d, pk_d, bk_sb, kt, True)
                qin0 = emit_kq_proj(0, wq_d, wq_sb, True, xq_d, pq_d, bq_sb,
                                    qt, False)

                # ---- V: DMA + adds now; PE chains deferred into the
                # attention stream as fillers (psout ring, 2 live accs) ----
                vins = []
                for half in range(2):
                    sl = slice(half * SH, (half + 1) * SH)
                    vin = lin_pool.tile([P, KC, SH], bf16, tag="lin")
                    for g0, g1 in GROUPS:
                        gs = slice(g0 * P, g1 * P)
                        if half == 0:
                            nc.sync.dma_start(wv_sb[:, g0:g1, :], wv_d[gs, :])
                        pr = raw_pool.tile([P, 4, SH], bf16, tag="raw")
                        nc.sync.dma_start(pr[:, 0:g1 - g0, :], pv_d[gs, sl])
                        for c in range(g0, g1):
                            nc.vector.tensor_add(
                                vin[:, c, :], xkv_sb[:, c, sl],
                                pr[:, c - g0, :])
                    vins.append(vin)

                # ---- Q-half1: DMA + adds now, PE chains deferred ----
                qin1 = lin_pool.tile([P, KC, SH], bf16, tag="lin")
                for g0, g1 in GROUPS:
                    gs = slice(g0 * P, g1 * P)
                    pr = raw_pool.tile([P, 4, SH], bf16, tag="raw")
                    xr = raw_pool.tile([P, 4, SH], bf16, tag="raw")
                    nc.sync.dma_start(pr[:, 0:g1 - g0, :], pq_d[gs, SH:2 * SH])
                    nc.sync.dma_start(xr[:, 0:g1 - g0, :], xq_d[gs, SH:2 * SH])
                    for c in range(g0, g1):
                        nc.vector.tensor_add(
                            qin1[:, c, :], xr[:, c - g0, :], pr[:, c - g0, :])
                nc.sync.dma_start(
                    wo_sb[:], wo_d[:].rearrange("(k p) f -> p k f", p=P))

                # ---- deferred PE chains (fillers) ----
                vdone = [0]

                def emit_vchain(half, sb):
                    vdone[0] += 1
                    acc = psout.tile([P, SH], f32, tag="ps")
                    vin = vins[half]
                    for c in range(KC):
                        nc.tensor.matmul(
                            acc[:],
                            vin[:, c, sb * P:(sb + 1) * P],
                            wv_sb[:, c, :],
                            start=(c == 0), stop=(c == KC - 1))
                    sc = half * 4 + sb
                    nc.vector.tensor_scalar(
                        vsb[:, sc, :, 0:HD],
                        acc[:].rearrange("p (h d) -> p h d", d=HD),
                        mk_sb[:, sc:sc + 1], None,
                        op0=mybir.AluOpType.mult)

                def emit_qh1_chain(m):
                    acc = psout.tile([P, SH], f32, tag="ps")
                    for c in range(KC):
                        nc.tensor.matmul(
                            acc[:],
                            wq_sb[:, c, m * P:(m + 1) * P],
                            qin1[:, c, :],
                            start=(c == 0), stop=(c == KC - 1))
                    nc.vector.tensor_scalar_add(
                        qt[:, m, SH:2 * SH], acc[:], bq_sb[:, m:m + 1])

                def emit_outchain(sh, m):
                    acc = psout.tile([P, SH], f32, tag="ps")
                    for hp in range(4):
                        nc.tensor.matmul(
                            acc[:],
                            wo_sb[:, hp, m * P:(m + 1) * P],
                            xt[:, hp, sh * SH:(sh + 1) * SH],
                            start=(hp == 0), stop=(hp == 3))
                    ob = out_pool.tile([P, SH], f32, tag="outb")
                    nc.vector.tensor_scalar_add(ob[:], acc[:], bo_sb[:, m:m + 1])
                    nc.sync.dma_start(
                        out_d[m * P:(m + 1) * P, sh * SH:(sh + 1) * SH], ob[:])


                # filler queue: (min_slot_index, emit_fn)
                fillers = [
                    (1, lambda: emit_vchain(0, 0)),
                    (2, lambda: emit_vchain(0, 1)),
                    (2, lambda: emit_vchain(0, 2)),
                    (3, lambda: emit_vchain(0, 3)),
                    (3, lambda: emit_vchain(1, 0)),
                    (4, lambda: emit_vchain(1, 1)),
                    (4, lambda: emit_vchain(1, 2)),
                    (5, lambda: emit_vchain(1, 3)),
                    (5, lambda: emit_qh1_chain(0)),
                    (6, lambda: emit_qh1_chain(1)),
                    (6, lambda: emit_qh1_chain(2)),
                    (7, lambda: emit_qh1_chain(3)),
                ]
                # out-h0 chains additionally require every sh=0 AV (heads
                # 0..7) to be emitted, so xt half0 is fully written.
                fillers += [(s, lambda m=m: emit_outchain(0, m), 8)
                            for m, s in enumerate((9, 10, 11, 12, 13, 13, 14, 14))]

                def pop_filler(si):
                    if not fillers or si < fillers[0][0]:
                        return False
                    if len(fillers[0]) > 2 and done_av < fillers[0][2]:
                        return False
                    fillers.pop(0)[1]()
                    return True

                # ---- attention slots, software-pipelined ----
                def emit_qk_pair(sh, h, pt, j):
                    """two s_k chunks (2j, 2j+1) of logits + one exp op."""
                    po = (h % 2) * HD
                    mq = h // 2
                    lg = pslg.tile([P, 2, SH], f32, tag="lg")
                    for i in range(2):
                        c = 2 * j + i
                        nc.tensor.matmul(
                            lg[:, i, :],
                            kt[po:po + HD, mq, c * P:(c + 1) * P],
                            qt[po:po + HD, mq, sh * SH:(sh + 1) * SH],
                            start=True, stop=True)
                    nc.scalar.activation(
                        pt[:, 2 * j:2 * j + 2, :], lg[:],
                        mybir.ActivationFunctionType.Exp)

                def emit_av_mm(h, pt):
                    av = psav.tile([P, 4, P], f32, tag="av")
                    for qb in range(4):
                        for c in range(SC):
                            nc.tensor.matmul(
                                av[:, qb, 0:HD1],
                                pt[:, c, qb * P:(qb + 1) * P],
                                vsb[:, c, h, :],
                                start=(c == 0), stop=(c == SC - 1))
                    return av

                def emit_av_norm(sh, h, av):
                    po = (h % 2) * HD
                    mq = h // 2
                    r = r_pool.tile([P, 4], f32, tag="r")
                    nc.vector.reciprocal(r[:], av[:, :, HD])
                    xn = xn_pool.tile([P, 4, HD], bf16, tag="xn")
                    for qb in range(4):
                        nc.vector.tensor_scalar(
                            xn[:, qb, :], av[:, qb, 0:HD], r[:, qb:qb + 1],
                            None, op0=mybir.AluOpType.mult)
                    xtT = pstr.tile([HD, 4, P], bf16, tag="xtT")
                    for qb in range(4):
                        nc.tensor.transpose(
                            xtT[0:HD, qb, :], xn[:, qb, :], ident[:])
                    nc.vector.tensor_copy(
                        xt[po:po + HD, mq, sh * SH:(sh + 1) * SH],
                        xtT[0:HD, :, :].rearrange("p a b -> p (a b)"))

                def emit_av(sh, h, pt):
                    emit_av_norm(sh, h, emit_av_mm(h, pt))

                # AV work is deferred until V (a filler stream) completes at
                # ~slot 5, then catches up at 2 AVs/slot; the pt ring (7 bufs)
                # covers the lag.
                slots = [(sh, h) for sh in range(2) for h in range(NH)]
                pend = []
                done_av = 0

                def emit_next_av(si):
                    nonlocal done_av
                    if (si >= 6 and vdone[0] == 8 and done_av < len(pend)
                            and done_av < si):
                        emit_av(*pend[done_av])
                        done_av += 1

                for si, (sh, h) in enumerate(slots):
                    pt = pt_pool.tile([P, SC, SH], bf16, tag="pbuf")
                    emit_qk_pair(sh, h, pt, 0)
                    emit_qk_pair(sh, h, pt, 1)
                    emit_next_av(si)
                    pop_filler(si)
                    emit_qk_pair(sh, h, pt, 2)
                    emit_qk_pair(sh, h, pt, 3)
                    if si == len(slots) - 1:
                        while done_av < len(pend):
                            emit_av(*pend[done_av])
                            done_av += 1
                    elif not pop_filler(si):
                        emit_next_av(si)
                    pend.append((sh, h, pt))
                while fillers:
                    fillers.pop(0)[1]()
                sh, h, pt = pend[-1]
                av_last = emit_av_mm(h, pt)
                accA = pslg.tile([P, 2, SH], f32, tag="lg")
                accB = pslg.tile([P, 2, SH], f32, tag="lg")
                acc_p0 = psout.tile([P, SH], f32, tag="ps")
                acc_p1 = psout.tile([P, SH], f32, tag="ps")
                accs6 = [acc_p0, acc_p1,
                         accA[:, 0, :], accA[:, 1, :],
                         accB[:, 0, :], accB[:, 1, :]]
                for m in range(6):
                    for hp in range(3):
                        nc.tensor.matmul(
                            accs6[m][:], wo_sb[:, hp, m * P:(m + 1) * P],
                            xt[:, hp, SH:2 * SH],
                            start=(hp == 0), stop=False)
                emit_av_norm(sh, h, av_last)
                # half-1 out-projection: plain chains, but biases land in
                # 4-chunk buffers so the tail DMAs go out 2 chunks at a time
                # (half the HWDGE generation serialization)
                ob4a = out4_pool.tile([P, 4, SH], f32, tag="outb4")
                ob4b = out4_pool.tile([P, 4, SH], f32, tag="outb4")
                ob4 = [ob4a, ob4b]
                for m in range(KC):
                    if m < 6:
                        nc.tensor.matmul(
                            accs6[m][:], wo_sb[:, 3, m * P:(m + 1) * P],
                            xt[:, 3, SH:2 * SH],
                            start=False, stop=True)
                        acc = accs6[m]
                    else:
                        acc = psout.tile([P, SH], f32, tag="ps")
                        for hp in range(4):
                            nc.tensor.matmul(
                                acc[:],
                                wo_sb[:, hp, m * P:(m + 1) * P],
                                xt[:, hp, SH:2 * SH],
                                start=(hp == 0), stop=(hp == 3))
                    if m % 2 == 0:
                        nc.vector.tensor_scalar_add(
                            ob4[m // 4][:, m % 4, :], acc[:], bo_sb[:, m:m + 1])
                    else:
                        nc.scalar.activation(
                            ob4[m // 4][:, m % 4, :], acc[:],
                            mybir.ActivationFunctionType.Identity,
                            bias=bo_sb[:, m:m + 1])
                    if m % 2 == 1:
                        nc.sync.dma_start(
                            out_d[(m - 1) * P:(m + 1) * P, SH:2 * SH]
                            .rearrange("(j p) s -> p j s", p=P),
                            ob4[m // 4][:, m % 4 - 1:m % 4 + 1, :])

    nc.compile()
    return nc


_program = None
_last_in_maps = None


def _get_program():
    global _program
    if _program is None:
        _program = build_program()
    return _program


def kernel(inputs_q, inputs_kv, pos_emb_q, pos_emb_k, pos_emb_v,
           key_padding_mask, wq, bq, wk, bk, wv, bv, wo, bo):
    nc = _get_program()

    bf = mybir.dt.np(bf16)

    wqf = np.asarray(wq, np.float32).reshape(D, H * HD)
    wkf = np.asarray(wk, np.float32).reshape(D, H * HD)
    wvf = np.asarray(wv, np.float32).reshape(D, H * HD)
    wof = np.asarray(wo, np.float32).reshape(H * HD, D)
    bqf = np.asarray(bq, np.float32).reshape(H * HD)
    bkf = np.asarray(bk, np.float32).reshape(H * HD)
    bvf = np.asarray(bv, np.float32).reshape(H * HD)
    bof = np.asarray(bo, np.float32).reshape(D)
    # bv is structurally zero in this problem; it has no cheap slot in the
    # transposed dataflow, so refuse loudly rather than silently drop it.
    assert np.all(bvf == 0.0), "nonzero bv is not supported"

    scale = np.float32(1.0 / np.sqrt(HD))
    iq = np.asarray(inputs_q, np.float32)
    ikv = np.asarray(inputs_kv, np.float32)
    pqa = np.asarray(pos_emb_q, np.float32)
    pka = np.asarray(pos_emb_k, np.float32)
    pva = np.asarray(pos_emb_v, np.float32)
    mask = np.asarray(key_padding_mask, np.float32)

    ident_np = np.eye(P, dtype=bf)

    in_maps = []
    for b in range(B):
        xq_t = np.ascontiguousarray(iq[b].T.astype(bf))
        xkv_t = np.ascontiguousarray(ikv[b].T.astype(bf))
        pq_t = np.ascontiguousarray(pqa[b].T.astype(bf))
        pk_t = np.ascontiguousarray(pka[b].T.astype(bf))
        pv_t = np.ascontiguousarray(pva[b].T.astype(bf))
        mk = np.ascontiguousarray(mask[b])
        # mask value per (partition, s-chunk, head) for V's denom column
        vones = np.ascontiguousarray(
            np.broadcast_to(mk.reshape(SC, P).T[:, :, None], (P, SC, NH))
        ).astype(bf)
        for hg in range(2):
            sl = slice(hg * F, (hg + 1) * F)
            in_maps.append({
                "xq": xq_t, "xkv": xkv_t, "pq": pq_t, "pk": pk_t, "pv": pv_t,
                "wq": np.ascontiguousarray(wqf[:, sl] * scale).astype(bf),
                "wk": np.ascontiguousarray(wkf[:, sl]).astype(bf),
                "wv": np.ascontiguousarray(wvf[:, sl]).astype(bf),
                "wo": np.ascontiguousarray(wof[sl, :]).astype(bf),
                "bq": np.ascontiguousarray(bqf[sl]) * scale,
                "bk": np.ascontiguousarray(bkf[sl]),
                "bo": bof if hg == 0 else np.zeros_like(bof),
                "mk": mk,
                "vones": vones,
                "ident": ident_np,
            })

    global _last_in_maps
    _last_in_maps = in_maps
    res = run_bass_kernel_spmd(nc, in_maps, list(range(2 * B)))
    outs = [res.results[i]["out_t"] for i in range(2 * B)]
    out = np.stack([(outs[2 * b] + outs[2 * b + 1]).T for b in range(B)])
    return np.ascontiguousarray(out, dtype=np.float32)


# revision 45
# speedup vs baseline: 1.0505x; 1.0314x over previous
# DETR multi-head dot-product attention for Trainium2 (Bass/Tile), 8 NeuronCores.
#
# Problem (hardcoded): B=4, S=1024, D=1024, H=16, HD=64, f32.
#   q = (inputs_q + pos_emb_q) @ wq + bq;  q /= sqrt(HD)
#   k = (inputs_kv + pos_emb_k) @ wk + bk
#   v = (inputs_kv + pos_emb_v) @ wv + bv          (bv == 0 by problem spec)
#   attn = softmax(q k^T + key_padding_bias); out = (attn v) @ wo + bo
#
# Sharding: 8 cores = 4 batches x 2 head-groups of 8 heads. Each core computes
# its batch's projections restricted to its head-group's features (512 of 1024),
# full attention for its 8 heads, and a partial output projection. The host
# sums the two head-group partials per batch.
#
# All matmul operands are bf16 (f32 PSUM accumulation); rel-err budget (2e-2)
# has ~10x headroom over bf16 noise. bf16 removes the f32r small-N penalty,
# which unlocks the cheap AV orientation: lhsT = P^T chunk [s_k,128 s_q],
# rhs = V chunk [s_k, HD+1] -> av[s_q, HD+1] at 65 cycles per chunk matmul
# (vs 512 the other way). Column HD of V is the padding mask, so av[:, HD]
# is the softmax denominator as a per-partition scalar: normalization is a
# reciprocal + per-partition tensor_scalar multiply, no broadcast matmul.
# A bf16 PE transpose restores feature-major x^T for the output projection;
# the Pool engine copies it PSUM->SBUF to keep DVE/ACT free.
#
# Schedule: projections accumulate over feature chunks in PSUM so the PE
# starts ~1.5us in, as soon as the first weight/input chunks land (the
# previous design waited ~22us for whole tensors). Phase order K, V, Q keeps
# the DMA queue aligned with consumption order; attention is software-
# pipelined across (s_q-half, head) slots with exp (ACT) as the pacing
# engine, and the Q-half1 projection + output projections are interleaved
# into the slot stream as PE filler.

import sys

for _p in ("/opt/trn_rl_repo", "/root/.axon_site/_ro/trn_rl_repo"):
    if _p not in sys.path:
        sys.path.append(_p)

import numpy as np

import concourse.bass as bass
import concourse.mybir as mybir
import concourse.tile as tile
from concourse import bacc
from concourse.bass_utils import run_bass_kernel_spmd

B, S, D = 4, 1024, 1024
H, HD = 16, 64
F = 512          # features per head-group core (8 heads * 64)
NH = 8           # heads per core
P = 128          # partitions
KC = D // P      # contraction chunks for the input projections (8)
SC = S // P      # sequence chunks (8)
SH = 512         # S-half
HD1 = HD + 1     # head_dim + denominator column

f32 = mybir.dt.float32
bf16 = mybir.dt.bfloat16


def build_program(repeat=1):
    nc = bacc.Bacc("TRN2", target_bir_lowering=False, debug=False)

    xq_d = nc.dram_tensor("xq", [D, S], bf16, kind="ExternalInput")
    xkv_d = nc.dram_tensor("xkv", [D, S], bf16, kind="ExternalInput")
    pq_d = nc.dram_tensor("pq", [D, S], bf16, kind="ExternalInput")
    pk_d = nc.dram_tensor("pk", [D, S], bf16, kind="ExternalInput")
    pv_d = nc.dram_tensor("pv", [D, S], bf16, kind="ExternalInput")
    wq_d = nc.dram_tensor("wq", [D, F], bf16, kind="ExternalInput")
    wk_d = nc.dram_tensor("wk", [D, F], bf16, kind="ExternalInput")
    wv_d = nc.dram_tensor("wv", [D, F], bf16, kind="ExternalInput")
    wo_d = nc.dram_tensor("wo", [F, D], bf16, kind="ExternalInput")
    bq_d = nc.dram_tensor("bq", [F], f32, kind="ExternalInput")
    bk_d = nc.dram_tensor("bk", [F], f32, kind="ExternalInput")
    bo_d = nc.dram_tensor("bo", [D], f32, kind="ExternalInput")
    mk_d = nc.dram_tensor("mk", [S], f32, kind="ExternalInput")  # padding mask
    # mask replicated per head for V's denominator column
    vones_d = nc.dram_tensor("vones", [P, SC, NH], bf16, kind="ExternalInput")
    ident_d = nc.dram_tensor("ident", [P, P], bf16, kind="ExternalInput")
    out_d = nc.dram_tensor("out_t", [D, S], f32, kind="ExternalOutput")

    with tile.TileContext(nc) as tc:
        with (
            tc.tile_pool(name="persist", bufs=1) as persist,
            tc.tile_pool(name="wmat", bufs=1) as w_pool,
            tc.tile_pool(name="lin", bufs=3) as lin_pool,
            tc.tile_pool(name="raw", bufs=5) as raw_pool,
            tc.tile_pool(name="ptp", bufs=7) as pt_pool,
            tc.tile_pool(name="xnp", bufs=2) as xn_pool,
            tc.tile_pool(name="rp", bufs=2) as r_pool,
            tc.tile_pool(name="outb", bufs=3) as out_pool,
            tc.tile_pool(name="outb4", bufs=2) as out4_pool,
            tc.tile_pool(name="pslg", bufs=2, space=bass.MemorySpace.PSUM) as pslg,
            tc.tile_pool(name="psav", bufs=1, space=bass.MemorySpace.PSUM) as psav,
            tc.tile_pool(name="pstr", bufs=1, space=bass.MemorySpace.PSUM) as pstr,
            tc.tile_pool(name="psout", bufs=2, space=bass.MemorySpace.PSUM) as psout,
        ):
            # ---- persistent tiles ----
            xkv_sb = persist.tile([P, KC, S], bf16, tag="xkv")
            kt = persist.tile([P, 4, S], bf16, tag="kt")     # K^T  [feature, s]
            qt = persist.tile([P, 4, S], bf16, tag="qt")     # Q^T
            xt = persist.tile([P, 4, S], bf16, tag="xt")     # attn-out^T, normalized
            # V natural layout [s, head, hd] + denominator column per head
            vsb = persist.tile([P, SC, NH, HD1], bf16, tag="vsb")
            bq_sb = persist.tile([P, 4], f32, tag="bq")
            bk_sb = persist.tile([P, 4], f32, tag="bk")
            bo_sb = persist.tile([P, KC], f32, tag="bo")
            mk_sb = persist.tile([P, SC], f32, tag="mk")
            ident = persist.tile([P, P], bf16, tag="ident")
            vst = persist.tile([P, SC, NH], bf16, tag="vst")

            wk_sb = w_pool.tile([P, KC, F], bf16, tag="wk")
            wv_sb = w_pool.tile([P, KC, F], bf16, tag="wv")
            wq_sb = w_pool.tile([P, KC, F], bf16, tag="wq")
            wo_sb = w_pool.tile([P, 4, D], bf16, tag="wo")

            # c-chunk DMA groups: small first chunks for an early PE start,
            # then bigger ones to amortize the per-DMA HWDGE generation cost.
            GROUPS = [(0, 2), (2, 4), (4, 8)]
            KGROUPS = [(0, 1), (1, 2), (2, 4), (4, 8)]

            for _rep in range(repeat):
                # small side-loads first (ACT queue; tiny transfers), so every
                # consumer below has its dependency edge in program order
                nc.scalar.dma_start(bk_sb[:], bk_d[:].rearrange("(m p) -> p m", p=P))
                nc.scalar.dma_start(bq_sb[:], bq_d[:].rearrange("(m p) -> p m", p=P))

                # ---- K / Q projection (feature-major out), c-accumulated ----
                def emit_kq_proj(half, w_d, w_sb, need_w, x_d, p_d, b_sb, dstT,
                                 from_xkv, groups=GROUPS):
                    sl = slice(half * SH, (half + 1) * SH)
                    accA = pslg.tile([P, 2, SH], f32, tag="lg")
                    accB = pslg.tile([P, 2, SH], f32, tag="lg")
                    accs = [accA[:, 0, :], accA[:, 1, :],
                            accB[:, 0, :], accB[:, 1, :]]
                    lin = lin_pool.tile([P, KC, SH], bf16, tag="lin")
                    for g0, g1 in groups:
                        gs = slice(g0 * P, g1 * P)
                        if need_w:
                            nc.sync.dma_start(w_sb[:, g0:g1, :], w_d[gs, :])
                        pr = raw_pool.tile([P, 4, SH], bf16, tag="raw")
                        nc.sync.dma_start(pr[:, 0:g1 - g0, :], p_d[gs, sl])
                        if from_xkv:
                            nc.sync.dma_start(xkv_sb[:, g0:g1, sl], x_d[gs, sl])
                        else:
                            xr = raw_pool.tile([P, 4, SH], bf16, tag="raw")
                            nc.sync.dma_start(xr[:, 0:g1 - g0, :], x_d[gs, sl])
                        for c in range(g0, g1):
                            if from_xkv:
                                nc.vector.tensor_add(
                                    lin[:, c, :], xkv_sb[:, c, sl],
                                    pr[:, c - g0, :])
                            else:
                                nc.vector.tensor_add(
                                    lin[:, c, :], xr[:, c - g0, :],
                                    pr[:, c - g0, :])
                            for m in range(4):
                                nc.tensor.matmul(
                                    accs[m],
                                    w_sb[:, c, m * P:(m + 1) * P],
                                    lin[:, c, :],
                                    start=(c == 0), stop=(c == KC - 1))
                    for m in range(4):
                        nc.vector.tensor_scalar_add(
                            dstT[:, m, sl], accs[m], b_sb[:, m:m + 1])
                    return lin

                emit_kq_proj(0, wk_d, wk_sb, True, xkv_d, pk_d, bk_sb, kt, True)
                emit_kq_proj(1, wk_d, wk_sb, False, xkv_d, pk_d, bk_sb, kt, True)
                qin0 = emit_kq_proj(0, wq_d, wq_sb, True, xq_d, pq_d, bq_sb,
                                    qt, False)
                nc.sync.dma_start(mk_sb[:], mk_d[:].rearrange("(c p) -> p c", p=P))
                nc.sync.dma_start(vst[:], vones_d[:])
                nc.sync.dma_start(ident[:], ident_d[:])
                nc.sync.dma_start(bo_sb[:], bo_d[:].rearrange("(m p) -> p m", p=P))
                nc.vector.tensor_copy(vsb[:, :, :, HD], vst[:])

                # ---- V: DMA + adds now; PE chains deferred into the
                # attention stream as fillers (psout ring, 2 live accs) ----
                vins = []
                for half in range(2):
                    sl = slice(half * SH, (half + 1) * SH)
                    vin = lin_pool.tile([P, KC, SH], bf16, tag="lin")
                    for g0, g1 in GROUPS:
                        gs = slice(g0 * P, g1 * P)
                        if half == 0:
                            nc.sync.dma_start(wv_sb[:, g0:g1, :], wv_d[gs, :])
                        pr = raw_pool.tile([P, 4, SH], bf16, tag="raw")
                        nc.sync.dma_start(pr[:, 0:g1 - g0, :], pv_d[gs, sl])
                        for c in range(g0, g1):
                            nc.vector.tensor_add(
                                vin[:, c, :], xkv_sb[:, c, sl],
                                pr[:, c - g0, :])
                    vins.append(vin)

                # ---- Q-half1: DMA + adds now, PE chains deferred ----
                qin1 = lin_pool.tile([P, KC, SH], bf16, tag="lin")
                for g0, g1 in GROUPS:
                    gs = slice(g0 * P, g1 * P)
                    pr = raw_pool.tile([P, 4, SH], bf16, tag="raw")
                    xr = raw_pool.tile([P, 4, SH], bf16, tag="raw")
                    nc.sync.dma_start(pr[:, 0:g1 - g0, :], pq_d[gs, SH:2 * SH])
                    nc.sync.dma_start(xr[:, 0:g1 - g0, :], xq_d[gs, SH:2 * SH])
                    for c in range(g0, g1):
                        nc.vector.tensor_add(
                            qin1[:, c, :], xr[:, c - g0, :], pr[:, c - g0, :])
                nc.sync.dma_start(
                    wo_sb[:], wo_d[:].rearrange("(k p) f -> p k f", p=P))

                # ---- deferred PE chains (fillers) ----
                vdone = [0]

                def emit_vchain(half, sb):
                    vdone[0] += 1
                    acc = psout.tile([P, SH], f32, tag="ps")
                    vin = vins[half]
                    for c in range(KC):
                        nc.tensor.matmul(
                            acc[:],
                            vin[:, c, sb * P:(sb + 1) * P],
                            wv_sb[:, c, :],
                            start=(c == 0), stop=(c == KC - 1))
                    sc = half * 4 + sb
                    nc.vector.tensor_scalar(
                        vsb[:, sc, :, 0:HD],
                        acc[:].rearrange("p (h d) -> p h d", d=HD),
                        mk_sb[:, sc:sc + 1], None,
                        op0=mybir.AluOpType.mult)

                def emit_qh1_chain(m):
                    acc = psout.tile([P, SH], f32, tag="ps")
                    for c in range(KC):
                        nc.tensor.matmul(
                            acc[:],
                            wq_sb[:, c, m * P:(m + 1) * P],
                            qin1[:, c, :],
                            start=(c == 0), stop=(c == KC - 1))
                    nc.vector.tensor_scalar_add(
                        qt[:, m, SH:2 * SH], acc[:], bq_sb[:, m:m + 1])

                def emit_outchain(sh, m):
                    acc = psout.tile([P, SH], f32, tag="ps")
                    for hp in range(4):
                        nc.tensor.matmul(
                            acc[:],
                            wo_sb[:, hp, m * P:(m + 1) * P],
                            xt[:, hp, sh * SH:(sh + 1) * SH],
                            start=(hp == 0), stop=(hp == 3))
                    ob = out_pool.tile([P, SH], f32, tag="outb")
                    nc.vector.tensor_scalar_add(ob[:], acc[:], bo_sb[:, m:m + 1])
                    nc.sync.dma_start(
                        out_d[m * P:(m + 1) * P, sh * SH:(sh + 1) * SH], ob[:])


                # filler queue: (min_slot_index, emit_fn)
                fillers = [
                    (1, lambda: emit_vchain(0, 0)),
                    (2, lambda: emit_vchain(0, 1)),
                    (2, lambda: emit_vchain(0, 2)),
                    (3, lambda: emit_vchain(0, 3)),
                    (3, lambda: emit_vchain(1, 0)),
                    (4, lambda: emit_vchain(1, 1)),
                    (4, lambda: emit_vchain(1, 2)),
                    (5, lambda: emit_vchain(1, 3)),
                    (5, lambda: emit_qh1_chain(0)),
                    (6, lambda: emit_qh1_chain(1)),
                    (6, lambda: emit_qh1_chain(2)),
                    (7, lambda: emit_qh1_chain(3)),
                ]
                # out-h0 chains additionally require every sh=0 AV (heads
                # 0..7) to be emitted, so xt half0 is fully written.
                fillers += [(s, lambda m=m: emit_outchain(0, m), 8)
                            for m, s in enumerate((9, 10, 11, 12, 13, 13, 14, 14))]

                def pop_filler(si):
                    if not fillers or si < fillers[0][0]:
                        return False
                    if len(fillers[0]) > 2 and done_av < fillers[0][2]:
                        return False
                    fillers.pop(0)[1]()
                    return True

                # ---- attention slots, software-pipelined ----
                def emit_qk_pair(sh, h, pt, j):
                    """two s_k chunks (2j, 2j+1) of logits + one exp op."""
                    po = (h % 2) * HD
                    mq = h // 2
                    lg = pslg.tile([P, 2, SH], f32, tag="lg")
                    for i in range(2):
                        c = 2 * j + i
                        nc.tensor.matmul(
                            lg[:, i, :],
                            kt[po:po + HD, mq, c * P:(c + 1) * P],
                            qt[po:po + HD, mq, sh * SH:(sh + 1) * SH],
                            start=True, stop=True)
                    nc.scalar.activation(
                        pt[:, 2 * j:2 * j + 2, :], lg[:],
                        mybir.ActivationFunctionType.Exp)

                def emit_av_mm(h, pt):
                    av = psav.tile([P, 4, P], f32, tag="av")
                    for qb in range(4):
                        for c in range(SC):
                            nc.tensor.matmul(
                                av[:, qb, 0:HD1],
                                pt[:, c, qb * P:(qb + 1) * P],
                                vsb[:, c, h, :],
                                start=(c == 0), stop=(c == SC - 1))
                    return av

                def emit_av_norm(sh, h, av):
                    po = (h % 2) * HD
                    mq = h // 2
                    r = r_pool.tile([P, 4], f32, tag="r")
                    nc.vector.reciprocal(r[:], av[:, :, HD])
                    xn = xn_pool.tile([P, 4, HD], bf16, tag="xn")
                    for qb in range(4):
                        nc.vector.tensor_scalar(
                            xn[:, qb, :], av[:, qb, 0:HD], r[:, qb:qb + 1],
                            None, op0=mybir.AluOpType.mult)
                    xtT = pstr.tile([HD, 4, P], bf16, tag="xtT")
                    for qb in range(4):
                        nc.tensor.transpose(
                            xtT[0:HD, qb, :], xn[:, qb, :], ident[:])
                    nc.vector.tensor_copy(
                        xt[po:po + HD, mq, sh * SH:(sh + 1) * SH],
                        xtT[0:HD, :, :].rearrange("p a b -> p (a b)"))

                def emit_av(sh, h, pt):
                    emit_av_norm(sh, h, emit_av_mm(h, pt))

                # AV work is deferred until V (a filler stream) completes at
                # ~slot 5, then catches up at 2 AVs/slot; the pt ring (7 bufs)
                # covers the lag.
                slots = [(sh, h) for sh in range(2) for h in range(NH)]
                pend = []
                done_av = 0

                def emit_next_av(si):
                    nonlocal done_av
                    if (si >= 6 and vdone[0] == 8 and done_av < len(pend)
                            and done_av < si):
                        emit_av(*pend[done_av])
                        done_av += 1

                for si, (sh, h) in enumerate(slots):
                    pt = pt_pool.tile([P, SC, SH], bf16, tag="pbuf")
                    emit_qk_pair(sh, h, pt, 0)
                    emit_qk_pair(sh, h, pt, 1)
                    emit_next_av(si)
                    pop_filler(si)
                    emit_qk_pair(sh, h, pt, 2)
                    emit_qk_pair(sh, h, pt, 3)
                    if si == len(slots) - 1:
                        while done_av < len(pend):
                            emit_av(*pend[done_av])
                            done_av += 1
                    elif not pop_filler(si):
                        emit_next_av(si)
                    pend.append((sh, h, pt))
                while fillers:
                    fillers.pop(0)[1]()
                sh, h, pt = pend[-1]
                av_last = emit_av_mm(h, pt)
                accA = pslg.tile([P, 2, SH], f32, tag="lg")
                accB = pslg.tile([P, 2, SH], f32, tag="lg")
                acc_p0 = psout.tile([P, SH], f32, tag="ps")
                acc_p1 = psout.tile([P, SH], f32, tag="ps")
                accs6 = [acc_p0, acc_p1,
                         accA[:, 0, :], accA[:, 1, :],
                         accB[:, 0, :], accB[:, 1, :]]
                for m in range(6):
                    for hp in range(3):
                        nc.tensor.matmul(
                            accs6[m][:], wo_sb[:, hp, m * P:(m + 1) * P],
                            xt[:, hp, SH:2 * SH],
                            start=(hp == 0), stop=False)
                emit_av_norm(sh, h, av_last)
                # half-1 out-projection: plain chains, but biases land in
                # 4-chunk buffers so the tail DMAs go out 2 chunks at a time
                # (half the HWDGE generation serialization)
                ob4a = out4_pool.tile([P, 4, SH], f32, tag="outb4")
                ob4b = out4_pool.tile([P, 4, SH], f32, tag="outb4")
                ob4 = [ob4a, ob4b]
                for m in range(KC):
                    if m < 6:
                        nc.tensor.matmul(
                            accs6[m][:], wo_sb[:, 3, m * P:(m + 1) * P],
                            xt[:, 3, SH:2 * SH],
                            start=False, stop=True)
                        acc = accs6[m]
                    else:
                        acc = psout.tile([P, SH], f32, tag="ps")
                        for hp in range(4):
                            nc.tensor.matmul(
                                acc[:],
                                wo_sb[:, hp, m * P:(m + 1) * P],
                                xt[:, hp, SH:2 * SH],
                                start=(hp == 0), stop=(hp == 3))
                    if m % 2 == 0:
                        nc.vector.tensor_scalar_add(
                            ob4[m // 4][:, m % 4, :], acc[:], bo_sb[:, m:m + 1])
                    else:
                        nc.scalar.activation(
                            ob4[m // 4][:, m % 4, :], acc[:],
                            mybir.ActivationFunctionType.Identity,
                            bias=bo_sb[:, m:m + 1])
                    if m % 2 == 1:
                        nc.sync.dma_start(
                            out_d[(m - 1) * P:(m + 1) * P, SH:2 * SH]
                            .rearrange("(j p) s -> p j s", p=P),
                            ob4[m // 4][:, m % 4 - 1:m % 4 + 1, :])

    nc.compile()
    return nc


_program = None
_last_in_maps = None


def _get_program():
    global _program
    if _program is None:
        _program = build_program()
    return _program


def kernel(inputs_q, inputs_kv, pos_emb_q, pos_emb_k, pos_emb_v,
           key_padding_mask, wq, bq, wk, bk, wv, bv, wo, bo):
    nc = _get_program()

    bf = mybir.dt.np(bf16)

    wqf = np.asarray(wq, np.float32).reshape(D, H * HD)
    wkf = np.asarray(wk, np.float32).reshape(D, H * HD)
    wvf = np.asarray(wv, np.float32).reshape(D, H * HD)
    wof = np.asarray(wo, np.float32).reshape(H * HD, D)
    bqf = np.asarray(bq, np.float32).reshape(H * HD)
    bkf = np.asarray(bk, np.float32).reshape(H * HD)
    bvf = np.asarray(bv, np.float32).reshape(H * HD)
    bof = np.asarray(bo, np.float32).reshape(D)
    # bv is structurally zero in this problem; it has no cheap slot in the
    # transposed dataflow, so refuse loudly rather than silently drop it.
    assert np.all(bvf == 0.0), "nonzero bv is not supported"

    scale = np.float32(1.0 / np.sqrt(HD))
    iq = np.asarray(inputs_q, np.float32)
    ikv = np.asarray(inputs_kv, np.float32)
    pqa = np.asarray(pos_emb_q, np.float32)
    pka = np.asarray(pos_emb_k, np.float32)
    pva = np.asarray(pos_emb_v, np.float32)
    mask = np.asarray(key_padding_mask, np.float32)

    ident_np = np.eye(P, dtype=bf)

    in_maps = []
    for b in range(B):
        xq_t = np.ascontiguousarray(iq[b].T.astype(bf))
        xkv_t = np.ascontiguousarray(ikv[b].T.astype(bf))
        pq_t = np.ascontiguousarray(pqa[b].T.astype(bf))
        pk_t = np.ascontiguousarray(pka[b].T.astype(bf))
        pv_t = np.ascontiguousarray(pva[b].T.astype(bf))
        mk = np.ascontiguousarray(mask[b])
        # mask value per (partition, s-chunk, head) for V's denom column
        vones = np.ascontiguousarray(
            np.broadcast_to(mk.reshape(SC, P).T[:, :, None], (P, SC, NH))
        ).astype(bf)
        for hg in range(2):
            sl = slice(hg * F, (hg + 1) * F)
            in_maps.append({
                "xq": xq_t, "xkv": xkv_t, "pq": pq_t, "pk": pk_t, "pv": pv_t,
                "wq": np.ascontiguousarray(wqf[:, sl] * scale).astype(bf),
                "wk": np.ascontiguousarray(wkf[:, sl]).astype(bf),
                "wv": np.ascontiguousarray(wvf[:, sl]).astype(bf),
                "wo": np.ascontiguousarray(wof[sl, :]).astype(bf),
                "bq": np.ascontiguousarray(bqf[sl]) * scale,
                "bk": np.ascontiguousarray(bkf[sl]),
                "bo": bof if hg == 0 else np.zeros_like(bof),
                "mk": mk,
                "vones": vones,
                "ident": ident_np,
            })

    global _last_in_maps
    _last_in_maps = in_maps
    res = run_bass_kernel_spmd(nc, in_maps, list(range(2 * B)))
    outs = [res.results[i]["out_t"] for i in range(2 * B)]
    out = np.stack([(outs[2 * b] + outs[2 * b + 1]).T for b in range(B)])
    return np.ascontiguousarray(out, dtype=np.float32)
